# revision 30
# baseline (speedup 1.0000x reference)
"""Distributed Trainium2 kernel for nn_Attention_7722351198977.

Math (reference):
    q,k,v = x@Wq, x@Wk, x@Wv          (B,T,H), B=8 T=1024 D=1024 H=64
    s = (q @ k^T) * sqrt(H)           causal mask BEFORE relpos bias
    s = where(tril, s, -inf) + einsum('btc,tvc->btv', q, relpos)
    out = softmax(s) @ v

Sharding: sequence-parallel over query time. Core i owns queries
t in [128*i, 128*(i+1)) for all batches. K/V are computed per-shard and
AllGather'd. relpos is pre-transposed host-side into "stacked pair"
tiles so the bias einsum becomes 64 dense matmuls per core:
    lhsT = block-diag q columns [128=2x64c, 16=(2t x 8b)]
    rhs  = [relpos[t0]^T ; relpos[t1]^T]  [128, 1024]
All per-core programs are shape-identical (SPMD-legal); only data differs.
"""

import os as _os

import ml_dtypes
import numpy as np

import concourse.bass as bass
import concourse.bacc as bacc
import concourse.mybir as mybir
import concourse.tile as tile
from concourse.bass_utils import run_bass_kernel_spmd
from concourse.masks import make_identity

F32 = mybir.dt.float32
BF16 = mybir.dt.bfloat16
B, T, D, H = 8, 1024, 1024, 64
NC = 8            # cores
TC = T // NC      # 128 queries per core
NPAIR = TC // 2   # 64 stacked pairs per core
MASK_VAL = -1.0e9

# matmul dtype knob: float32r streams fp32 at 1 cyc/row (N>=256) vs 4 for
# plain fp32. Must be validated on HW (probe) before enabling.
MM_F32R = _os.environ.get("ATTN_MM_F32R", "0") == "1"
PHASE = int(_os.environ.get("ATTN_PHASE", "9"))


def _mm(ap):
    return ap.bitcast(mybir.dt.float32r) if MM_F32R else ap


def build(num_cores: int = NC) -> bass.Bass:
    nc = bacc.Bacc(
        "TRN2", target_bir_lowering=False, debug=False, num_devices=num_cores
    )

    xT = nc.declare_dram_parameter("xT", [D, B * TC], F32, isOutput=False)
    wqk = nc.declare_dram_parameter("wqk", [D, 2 * H], F32, isOutput=False)
    wv = nc.declare_dram_parameter("wv", [D, H], F32, isOutput=False)
    relp = nc.declare_dram_parameter("relp", [NPAIR, TC, T], BF16, isOutput=False)
    mask = nc.declare_dram_parameter("mask", [TC, T], F32, isOutput=False)
    out_e = nc.declare_dram_parameter("out", [B * TC, H], F32, isOutput=True)

    stage_ds = [
        nc.dram_tensor(f"stg{hf}", [8, 8, 8, T], BF16) for hf in range(2)
    ]
    cc_in = nc.dram_tensor("cc_in", [TC, B * TC], F32)
    cc_out = nc.dram_tensor("cc_out", [NC * TC, B * TC], F32, addr_space="Shared")

    Copy = mybir.ActivationFunctionType.Copy
    Exp = mybir.ActivationFunctionType.Exp

    with tile.TileContext(nc) as tc:
        with (
            tc.tile_pool(name="const", bufs=1) as constp,
            tc.tile_pool(name="big", bufs=1) as bigp,
            tc.tile_pool(name="relps", bufs=3) as relpp,
            tc.tile_pool(name="attn", bufs=2) as attnp,
            tc.tile_pool(name="small", bufs=8) as smallp,
            tc.tile_pool(name="psc", bufs=2, space="PSUM") as ps_sc,
            tc.tile_pool(name="pat", bufs=2, space="PSUM") as ps_at,
            tc.tile_pool(name="pbias", bufs=1, space="PSUM") as ps_b,
        ):
            # ---- constants / inputs to SBUF ----
            ident = constp.tile([128, 128], F32)
            make_identity(nc, ident[:])
            ident_bf = constp.tile([64, 64], BF16)
            nc.vector.tensor_copy(ident_bf[:], ident[0:64, 0:64])
            zero_sb = constp.tile([128, 512], F32)
            nc.gpsimd.memset(zero_sb[:], 0.0)

            wqk_sb = constp.tile([128, 8, 2 * H], F32)
            nc.sync.dma_start(
                out=wqk_sb[:], in_=wqk.rearrange("(c p) m -> p c m", p=128)
            )
            wv_sb = constp.tile([128, 8, H], F32)
            nc.sync.dma_start(
                out=wv_sb[:], in_=wv.rearrange("(c p) m -> p c m", p=128)
            )
            mask_sb = constp.tile([64, 2, T], F32)
            nc.sync.dma_start(
                out=mask_sb[:], in_=mask.rearrange("(hf p) v -> p hf v", hf=2)
            )

            # persistent PSUM tiles for bias quads; double duty as the
            # projection accumulators before the bias phase starts.
            psB = [
                ps_b.tile([128, 512], F32, tag=f"psB{h}", name=f"psB{h}")
                for h in range(2)
            ]

            with tc.tile_pool(name="xtp", bufs=1) as xtp:
                xT_sb = xtp.tile([128, 8, B * TC], F32)
                nc.sync.dma_start(
                    out=xT_sb[:], in_=xT.rearrange("(c p) r -> p c r", p=128)
                )

                # ---- PASS A: qT | kT projection (one stream, M=128) ----
                for h2 in range(2):
                    for c in range(8):
                        nc.tensor.matmul(
                            psB[h2][:],
                            _mm(wqk_sb[:, c, :]),
                            _mm(xT_sb[:, c, h2 * 512 : (h2 + 1) * 512]),
                            start=(c == 0),
                            stop=(c == 7),
                        )
                qT_sb = constp.tile([H, B * TC], F32)
                kT_loc = constp.tile([H, B * TC], F32)
                for h2 in range(2):
                    sl = slice(h2 * 512, (h2 + 1) * 512)
                    nc.scalar.activation(qT_sb[:, sl], psB[h2][0:H, :], Copy)
                    # fold the sqrt(H)=8 score scale into k
                    nc.scalar.activation(
                        kT_loc[:, sl], psB[h2][H:128, :], Copy, scale=8.0
                    )

                # ---- PASS B: v natural ([tl, (b,h)]) per batch ----
                for b in range(B):
                    for c in range(8):
                        nc.tensor.matmul(
                            psB[0][:, b * H : (b + 1) * H],
                            _mm(xT_sb[:, c, b * TC : (b + 1) * TC]),
                            _mm(wv_sb[:, c, :]),
                            start=(c == 0),
                            stop=(c == 7),
                        )
                v_loc = constp.tile([128, B * H], F32)
                nc.scalar.activation(v_loc[:], psB[0][:], Copy)

            # ---- AllGather k,v ----
            if PHASE >= 2:
                nc.sync.dma_start(out=cc_in[0:H, :], in_=kT_loc[:])
                nc.sync.dma_start(
                    out=cc_in[H:TC, :].rearrange("p (a c) -> (p a) c", a=2),
                    in_=v_loc[:],
                )
                nc.gpsimd.collective_compute(
                    "AllGather",
                    mybir.AluOpType.bypass,
                    replica_groups=[list(range(num_cores))],
                    ins=[cc_in[:]],
                    outs=[cc_out[:]],
                )
                kT_nat = bigp.tile([H, B, NC, TC], F32)  # [64, (b, j, tl)]
                nc.sync.dma_start(
                    out=kT_nat[:],
                    in_=cc_out.rearrange(
                        "(j tw p) (b t) -> tw p b j t", j=NC, tw=2, p=H, b=B
                    )[0],
                )
                v_nat = bigp.tile([128, B, NC, H], BF16)  # [tl, (b, j, h)]
                nc.gpsimd.dma_start(
                    out=v_nat[:],
                    in_=cc_out.rearrange(
                        "(j tw th) (tp b h) -> tw (th tp) b j h",
                        j=NC, tw=2, th=64, tp=2, b=B, h=H,
                    )[1],
                )

            # ---- block-diag q stage for bias matmuls ----
            qstage = constp.tile([128, NPAIR * 16], BF16)
            nc.gpsimd.memset(qstage[:], 0.0)
            qsrc = qT_sb.rearrange("c (b pp s) -> c pp s b", b=B, pp=NPAIR, s=2)
            qdst_lo = qstage[0:64, :].rearrange(
                "c (pp s b) -> c pp s b", pp=NPAIR, s=2, b=B
            )
            qdst_hi = qstage[64:128, :].rearrange(
                "c (pp s b) -> c pp s b", pp=NPAIR, s=2, b=B
            )
            nc.vector.tensor_copy(qdst_lo[:, :, 0, :], qsrc[:, :, 0, :])
            nc.vector.tensor_copy(qdst_hi[:, :, 1, :], qsrc[:, :, 1, :])

            # zero-init the bias psum tiles (only partition strips of each
            # 32-block get written by the M=16 matmuls; the eviction reads
            # all 128 partitions).
            for h2 in range(2):
                nc.scalar.activation(psB[h2][:], zero_sb[:], Copy)

            # ---- mask prefill of bias tables, then bias pairs + scatter ----
            bias_tc = [
                bigp.tile([64, B * T], BF16, tag=f"btc{c}", name=f"bias_tc{c}")
                for c in range(2)
            ]
            bias_sb = bigp.tile([128, 8, T], BF16)  # per half: 32 pairs
            for half in range(2 if PHASE >= 3 else 0):
                for q in range(32):
                    p = half * 32 + q
                    quad, pm4 = q // 4, q % 4
                    relp_t = relpp.tile([TC, T], BF16, tag="rp")
                    dma_eng = nc.sync if p % 2 == 0 else nc.scalar
                    dma_eng.dma_start(out=relp_t[:], in_=relp[p])
                    for h2 in range(2):
                        nc.tensor.matmul(
                            psB[h2][32 * pm4 : 32 * pm4 + 16, :],
                            _mm(qstage[:, 16 * p : 16 * p + 16]),
                            _mm(relp_t[:, h2 * 512 : (h2 + 1) * 512]),
                            tile_position=(0, 32 * pm4),
                            start=True,
                            stop=True,
                        )
                    if pm4 == 3:
                        for h2 in range(2):
                            sl = slice(h2 * 512, (h2 + 1) * 512)
                            nc.scalar.activation(
                                bias_sb[:, quad, sl], psB[h2][:], Copy
                            )
                # rearrange to [tl, (b, v)] via a DRAM staging hop:
                # write (pm4,ts)-strips linearly, read back with the
                # scatter on the DRAM side (no SBUF partition constraints).
                for pm4 in range(4):
                    for ts in range(2):
                        pt = 2 * pm4 + ts
                        r0 = 32 * pm4 + 8 * ts
                        nc.sync.dma_start(
                            out=stage_ds[half][pt],
                            in_=bias_sb[r0 : r0 + 8, :, :],
                        )
                for q in range(8):
                    nc.sync.dma_start(
                        out=bias_tc[half][8 * q : 8 * q + 8, :],
                        in_=stage_ds[half][:, :, q, :],
                    )
                # fold causal mask in-place (bias_tc += mask rows)
                for b in range(B):
                    nc.vector.tensor_tensor(
                        out=bias_tc[half][:, b * T : (b + 1) * T],
                        in0=bias_tc[half][:, b * T : (b + 1) * T],
                        in1=mask_sb[:, half, :],
                        op=mybir.AluOpType.add,
                    )

            # ---- per (batch, 64-query chunk): scores -> softmax -> @v ----
            for ch in range(2 if PHASE >= 4 else 0):
                for b in range(B):
                    psS0 = ps_sc.tile([64, 512], F32, tag="sc0", name="psS0")
                    psS1 = ps_sc.tile([64, 512], F32, tag="sc1", name="psS1")
                    psS = [psS0, psS1]
                    lhs_q = qT_sb[:, b * TC + 64 * ch : b * TC + 64 * ch + 64]
                    for h2 in range(2):
                        nc.tensor.matmul(
                            psS[h2][:],
                            _mm(lhs_q),
                            _mm(kT_nat[:, b, 4 * h2 : 4 * h2 + 4, :]),
                            start=True,
                            stop=True,
                        )
                    attn_pre = attnp.tile([64, T], F32, tag="apre")
                    for h2 in range(2):
                        sl = slice(h2 * 512, (h2 + 1) * 512)
                        nc.vector.tensor_tensor(
                            out=attn_pre[:, sl],
                            in0=psS[h2][:],
                            in1=bias_tc[ch][
                                :, b * T + h2 * 512 : b * T + (h2 + 1) * 512
                            ],
                            op=mybir.AluOpType.add,
                        )
                    if PHASE < 5:
                        continue
                    negmax = smallp.tile([64, 1], F32, tag="nmax")
                    nc.vector.reduce_max(
                        negmax[:], attn_pre[:], axis=mybir.AxisListType.X,
                        negate=True,
                    )
                    attn_e = attnp.tile([64, T], BF16, tag="aexp")
                    denom = smallp.tile([64, 1], F32, tag="den")
                    nc.scalar.activation(
                        attn_e[:],
                        attn_pre[:],
                        Exp,
                        bias=negmax[:],
                        scale=1.0,
                        accum_out=denom[:],
                    )
                    if PHASE < 6:
                        continue
                    psT = ps_at.tile([128, 512], BF16, tag="at")
                    for s8 in range(8):
                        nc.tensor.transpose(
                            psT[:, 64 * s8 : 64 * s8 + 64],
                            attn_e[:, 128 * s8 : 128 * s8 + 128],
                            ident_bf[:],
                        )
                    attnT = attnp.tile([128, 512], BF16, tag="aT")
                    nc.scalar.activation(attnT[:], psT[:], Copy)
                    if PHASE < 7:
                        continue
                    psO = ps_at.tile([64, H], F32, tag="at")
                    for s8 in range(8):
                        nc.tensor.matmul(
                            psO[:],
                            _mm(attnT[:, 64 * s8 : 64 * s8 + 64]),
                            _mm(v_nat[:, b, s8, :]),
                            start=(s8 == 0),
                            stop=(s8 == 7),
                        )
                    rden = smallp.tile([64, 1], F32, tag="rden")
                    nc.vector.reciprocal(rden[:], denom[:])
                    out_sb = smallp.tile([64, H], F32, tag="osb")
                    nc.scalar.activation(out_sb[:], psO[:], Copy, scale=rden[:])
                    r0 = b * TC + 64 * ch
                    nc.sync.dma_start(out=out_e[r0 : r0 + 64, :], in_=out_sb[:])
            if PHASE < 9:
                dummy = smallp.tile([64, H], F32, tag="osb")
                nc.vector.tensor_copy(dummy[:], qT_sb[:, 0:H])
                for r in range(0, B * TC, 64):
                    nc.sync.dma_start(out=out_e[r : r + 64, :], in_=dummy[:])
    nc.compile()
    return nc


_CACHE: dict = {}


def _get_nc():
    if "nc" not in _CACHE:
        _CACHE["nc"] = build(NC)
    return _CACHE["nc"]


def _prep_inputs(x, Wq, Wk, Wv, relpos):
    x = np.ascontiguousarray(x, dtype=np.float32)
    relpos = np.ascontiguousarray(relpos, dtype=np.float32)
    wqk = np.ascontiguousarray(
        np.concatenate([Wq, Wk], axis=1), dtype=np.float32
    )
    wv = np.ascontiguousarray(Wv, dtype=np.float32)
    in_maps = []
    for i in range(NC):
        xs = x[:, TC * i : TC * (i + 1), :]            # (B, TC, D)
        xT = np.ascontiguousarray(
            xs.transpose(2, 0, 1).reshape(D, B * TC)
        )
        rp = relpos[TC * i : TC * (i + 1)]             # (TC, T, H)
        relp = np.ascontiguousarray(
            rp.transpose(0, 2, 1).reshape(NPAIR, TC, T)
        ).astype(ml_dtypes.bfloat16)
        tl = np.arange(TC)[:, None]
        vv = np.arange(T)[None, :]
        msk = np.where(vv <= TC * i + tl, 0.0, MASK_VAL).astype(np.float32)
        in_maps.append(
            {"xT": xT, "wqk": wqk, "wv": wv, "relp": relp, "mask": msk}
        )
    return in_maps


def run_sharded(in_maps, trace=False, **kw):
    nc = _get_nc()
    return run_bass_kernel_spmd(
        nc, in_maps, core_ids=list(range(NC)), trace=trace, **kw
    )


def kernel(x, Wq, Wk, Wv, relpos):
    in_maps = _prep_inputs(x, Wq, Wk, Wv, relpos)
    res = run_sharded(in_maps, trace=False)
    out = np.empty((B, T, H), dtype=np.float32)
    for i in range(NC):
        out[:, TC * i : TC * (i + 1), :] = (
            res.results[i]["out"].reshape(B, TC, H)
        )
    return out


# revision 31
# speedup vs baseline: 1.2598x; 1.2598x over previous
"""Distributed Trainium2 kernel for nn_Attention_7722351198977.

Math (reference):
    q,k,v = x@Wq, x@Wk, x@Wv          (B,T,H), B=8 T=1024 D=1024 H=64
    s = (q @ k^T) * sqrt(H)           causal mask BEFORE relpos bias
    s = where(tril, s, -inf) + einsum('btc,tvc->btv', q, relpos)
    out = softmax(s) @ v

Sharding: sequence-parallel over query time. Core i owns queries
t in [128*i, 128*(i+1)) for all batches. K/V are computed per-shard and
AllGather'd. relpos is pre-transposed host-side into "stacked pair"
tiles so the bias einsum becomes 64 dense matmuls per core:
    lhsT = block-diag q columns [128=2x64c, 16=(2t x 8b)]
    rhs  = [relpos[t0]^T ; relpos[t1]^T]  [128, 1024]
All per-core programs are shape-identical (SPMD-legal); only data differs.
"""

import os as _os

import ml_dtypes
import numpy as np

import concourse.bass as bass
import concourse.bacc as bacc
import concourse.mybir as mybir
import concourse.tile as tile
from concourse.bass_utils import run_bass_kernel_spmd
from concourse.masks import make_identity

F32 = mybir.dt.float32
BF16 = mybir.dt.bfloat16
B, T, D, H = 8, 1024, 1024, 64
NC = 8            # cores
TC = T // NC      # 128 queries per core
NPAIR = TC // 2   # 64 stacked pairs per core
MASK_VAL = -1.0e9

# matmul dtype knob: float32r streams fp32 at 1 cyc/row (N>=256) vs 4 for
# plain fp32. Must be validated on HW (probe) before enabling.
MM_F32R = _os.environ.get("ATTN_MM_F32R", "0") == "1"
PHASE = int(_os.environ.get("ATTN_PHASE", "9"))


def _mm(ap):
    return ap.bitcast(mybir.dt.float32r) if MM_F32R else ap


def build(num_cores: int = NC) -> bass.Bass:
    nc = bacc.Bacc(
        "TRN2", target_bir_lowering=False, debug=False, num_devices=num_cores
    )

    xT = nc.declare_dram_parameter("xT", [D, B * TC], F32, isOutput=False)
    wqk = nc.declare_dram_parameter("wqk", [D, 2 * H], F32, isOutput=False)
    wv = nc.declare_dram_parameter("wv", [D, H], F32, isOutput=False)
    relp = nc.declare_dram_parameter("relp", [NPAIR, TC, T], BF16, isOutput=False)
    mask = nc.declare_dram_parameter("mask", [TC, T], F32, isOutput=False)
    out_e = nc.declare_dram_parameter("out", [B * TC, H], F32, isOutput=True)

    stage_ds = [
        nc.dram_tensor(f"stg{hf}", [8, 8, 8, T], BF16) for hf in range(2)
    ]
    cc_in = nc.dram_tensor("cc_in", [TC, B * TC], F32)
    cc_out = nc.dram_tensor("cc_out", [NC * TC, B * TC], F32, addr_space="Shared")

    Copy = mybir.ActivationFunctionType.Copy
    Exp = mybir.ActivationFunctionType.Exp

    with tile.TileContext(nc) as tc:
        with (
            tc.tile_pool(name="const", bufs=1) as constp,
            tc.tile_pool(name="big", bufs=1) as bigp,
            tc.tile_pool(name="relps", bufs=6) as relpp,
            tc.tile_pool(name="attn", bufs=2) as attnp,
            tc.tile_pool(name="small", bufs=8) as smallp,
            tc.tile_pool(name="psc", bufs=2, space="PSUM") as ps_sc,
            tc.tile_pool(name="pat", bufs=2, space="PSUM") as ps_at,
            tc.tile_pool(name="pbias", bufs=1, space="PSUM") as ps_b,
        ):
            # ---- constants / inputs to SBUF ----
            ident = constp.tile([128, 128], F32)
            make_identity(nc, ident[:])
            ident_bf = constp.tile([64, 64], BF16)
            nc.vector.tensor_copy(ident_bf[:], ident[0:64, 0:64])
            zero_sb = constp.tile([128, 512], F32)
            nc.gpsimd.memset(zero_sb[:], 0.0)

            wqk_sb = constp.tile([128, 8, 2 * H], F32)
            nc.sync.dma_start(
                out=wqk_sb[:], in_=wqk.rearrange("(c p) m -> p c m", p=128)
            )
            wv_sb = constp.tile([128, 8, H], F32)
            nc.sync.dma_start(
                out=wv_sb[:], in_=wv.rearrange("(c p) m -> p c m", p=128)
            )
            mask_sb = constp.tile([64, 2, T], F32)
            nc.sync.dma_start(
                out=mask_sb[:], in_=mask.rearrange("(hf p) v -> p hf v", hf=2)
            )

            # persistent PSUM tiles for bias quads; double duty as the
            # projection accumulators before the bias phase starts.
            psB = [
                ps_b.tile([128, 512], F32, tag=f"psB{h}", name=f"psB{h}")
                for h in range(2)
            ]

            with tc.tile_pool(name="xtp", bufs=1) as xtp:
                xT_sb = xtp.tile([128, 8, B * TC], F32)
                nc.sync.dma_start(
                    out=xT_sb[:], in_=xT.rearrange("(c p) r -> p c r", p=128)
                )

                # ---- PASS A: qT | kT projection (one stream, M=128) ----
                for h2 in range(2):
                    for c in range(8):
                        nc.tensor.matmul(
                            psB[h2][:],
                            _mm(wqk_sb[:, c, :]),
                            _mm(xT_sb[:, c, h2 * 512 : (h2 + 1) * 512]),
                            start=(c == 0),
                            stop=(c == 7),
                        )
                qT_sb = constp.tile([H, B * TC], F32)
                kT_loc = constp.tile([H, B * TC], F32)
                for h2 in range(2):
                    sl = slice(h2 * 512, (h2 + 1) * 512)
                    nc.scalar.activation(qT_sb[:, sl], psB[h2][0:H, :], Copy)
                    # fold the sqrt(H)=8 score scale into k
                    nc.scalar.activation(
                        kT_loc[:, sl], psB[h2][H:128, :], Copy, scale=8.0
                    )

                # ---- PASS B: v natural ([tl, (b,h)]) per batch ----
                for b in range(B):
                    for c in range(8):
                        nc.tensor.matmul(
                            psB[0][:, b * H : (b + 1) * H],
                            _mm(xT_sb[:, c, b * TC : (b + 1) * TC]),
                            _mm(wv_sb[:, c, :]),
                            start=(c == 0),
                            stop=(c == 7),
                        )
                v_loc = constp.tile([128, B * H], F32)
                nc.scalar.activation(v_loc[:], psB[0][:], Copy)

            # ---- AllGather k,v ----
            if PHASE >= 2:
                nc.sync.dma_start(out=cc_in[0:H, :], in_=kT_loc[:])
                nc.sync.dma_start(
                    out=cc_in[H:TC, :].rearrange("p (a c) -> (p a) c", a=2),
                    in_=v_loc[:],
                )
                nc.gpsimd.collective_compute(
                    "AllGather",
                    mybir.AluOpType.bypass,
                    replica_groups=[list(range(num_cores))],
                    ins=[cc_in[:]],
                    outs=[cc_out[:]],
                )
                kT_nat = bigp.tile([H, B, NC, TC], F32)  # [64, (b, j, tl)]
                nc.sync.dma_start(
                    out=kT_nat[:],
                    in_=cc_out.rearrange(
                        "(j tw p) (b t) -> tw p b j t", j=NC, tw=2, p=H, b=B
                    )[0],
                )
                v_nat = bigp.tile([128, B, NC, H], BF16)  # [tl, (b, j, h)]
                nc.gpsimd.dma_start(
                    out=v_nat[:],
                    in_=cc_out.rearrange(
                        "(j tw th) (tp b h) -> tw (th tp) b j h",
                        j=NC, tw=2, th=64, tp=2, b=B, h=H,
                    )[1],
                )

            # ---- block-diag q stage for bias matmuls ----
            qstage = constp.tile([128, NPAIR * 16], BF16)
            nc.gpsimd.memset(qstage[:], 0.0)
            qsrc = qT_sb.rearrange("c (b pp s) -> c pp s b", b=B, pp=NPAIR, s=2)
            qdst_lo = qstage[0:64, :].rearrange(
                "c (pp s b) -> c pp s b", pp=NPAIR, s=2, b=B
            )
            qdst_hi = qstage[64:128, :].rearrange(
                "c (pp s b) -> c pp s b", pp=NPAIR, s=2, b=B
            )
            nc.vector.tensor_copy(qdst_lo[:, :, 0, :], qsrc[:, :, 0, :])
            nc.vector.tensor_copy(qdst_hi[:, :, 1, :], qsrc[:, :, 1, :])

            # zero-init the bias psum tiles (only partition strips of each
            # 32-block get written by the M=16 matmuls; the eviction reads
            # all 128 partitions).
            for h2 in range(2):
                nc.scalar.activation(psB[h2][:], zero_sb[:], Copy)

            # ---- mask prefill of bias tables, then bias pairs + scatter ----
            bias_tc = [
                bigp.tile([64, B * T], BF16, tag=f"btc{c}", name=f"bias_tc{c}")
                for c in range(2)
            ]
            bias_sb = bigp.tile([128, 8, T], BF16)  # per half: 32 pairs
            for half in range(2 if PHASE >= 3 else 0):
                for q in range(32):
                    p = half * 32 + q
                    quad, pm4 = q // 4, q % 4
                    relp_t = relpp.tile([TC, T], BF16, tag="rp")
                    dma_eng = nc.sync if p % 2 == 0 else nc.scalar
                    dma_eng.dma_start(out=relp_t[:], in_=relp[p])
                    for h2 in range(2):
                        nc.tensor.matmul(
                            psB[h2][32 * pm4 : 32 * pm4 + 16, :],
                            _mm(qstage[:, 16 * p : 16 * p + 16]),
                            _mm(relp_t[:, h2 * 512 : (h2 + 1) * 512]),
                            tile_position=(0, 32 * pm4),
                            start=True,
                            stop=True,
                        )
                    if pm4 == 3:
                        for h2 in range(2):
                            sl = slice(h2 * 512, (h2 + 1) * 512)
                            nc.scalar.activation(
                                bias_sb[:, quad, sl], psB[h2][:], Copy
                            )
                # rearrange to [tl, (b, v)] via a DRAM staging hop:
                # write (pm4,ts)-strips linearly, read back with the
                # scatter on the DRAM side (no SBUF partition constraints).
                for pm4 in range(4):
                    for ts in range(2):
                        pt = 2 * pm4 + ts
                        r0 = 32 * pm4 + 8 * ts
                        nc.sync.dma_start(
                            out=stage_ds[half][pt],
                            in_=bias_sb[r0 : r0 + 8, :, :],
                        )
                for q in range(8):
                    nc.sync.dma_start(
                        out=bias_tc[half][8 * q : 8 * q + 8, :],
                        in_=stage_ds[half][:, :, q, :],
                    )
                # fold causal mask in-place (bias_tc += mask rows)
                for b in range(B):
                    nc.vector.tensor_tensor(
                        out=bias_tc[half][:, b * T : (b + 1) * T],
                        in0=bias_tc[half][:, b * T : (b + 1) * T],
                        in1=mask_sb[:, half, :],
                        op=mybir.AluOpType.add,
                    )

            # ---- per (batch, 64-query chunk): scores -> softmax -> @v ----
            for ch in range(2 if PHASE >= 4 else 0):
                for b in range(B):
                    psS0 = ps_sc.tile([64, 512], F32, tag="sc0", name="psS0")
                    psS1 = ps_sc.tile([64, 512], F32, tag="sc1", name="psS1")
                    psS = [psS0, psS1]
                    lhs_q = qT_sb[:, b * TC + 64 * ch : b * TC + 64 * ch + 64]
                    for h2 in range(2):
                        nc.tensor.matmul(
                            psS[h2][:],
                            _mm(lhs_q),
                            _mm(kT_nat[:, b, 4 * h2 : 4 * h2 + 4, :]),
                            start=True,
                            stop=True,
                        )
                    attn_pre = attnp.tile([64, T], F32, tag="apre")
                    for h2 in range(2):
                        sl = slice(h2 * 512, (h2 + 1) * 512)
                        nc.vector.tensor_tensor(
                            out=attn_pre[:, sl],
                            in0=psS[h2][:],
                            in1=bias_tc[ch][
                                :, b * T + h2 * 512 : b * T + (h2 + 1) * 512
                            ],
                            op=mybir.AluOpType.add,
                        )
                    if PHASE < 5:
                        continue
                    negmax = smallp.tile([64, 1], F32, tag="nmax")
                    nc.vector.reduce_max(
                        negmax[:], attn_pre[:], axis=mybir.AxisListType.X,
                        negate=True,
                    )
                    attn_e = attnp.tile([64, T], BF16, tag="aexp")
                    denom = smallp.tile([64, 1], F32, tag="den")
                    nc.scalar.activation(
                        attn_e[:],
                        attn_pre[:],
                        Exp,
                        bias=negmax[:],
                        scale=1.0,
                        accum_out=denom[:],
                    )
                    if PHASE < 6:
                        continue
                    psT = ps_at.tile([128, 512], BF16, tag="at")
                    for s8 in range(8):
                        nc.tensor.transpose(
                            psT[:, 64 * s8 : 64 * s8 + 64],
                            attn_e[:, 128 * s8 : 128 * s8 + 128],
                            ident_bf[:],
                        )
                    attnT = attnp.tile([128, 512], BF16, tag="aT")
                    nc.scalar.activation(attnT[:], psT[:], Copy)
                    if PHASE < 7:
                        continue
                    psO = ps_at.tile([64, H], F32, tag="at")
                    for s8 in range(8):
                        nc.tensor.matmul(
                            psO[:],
                            _mm(attnT[:, 64 * s8 : 64 * s8 + 64]),
                            _mm(v_nat[:, b, s8, :]),
                            start=(s8 == 0),
                            stop=(s8 == 7),
                        )
                    rden = smallp.tile([64, 1], F32, tag="rden")
                    nc.vector.reciprocal(rden[:], denom[:])
                    out_sb = smallp.tile([64, H], F32, tag="osb")
                    nc.scalar.activation(out_sb[:], psO[:], Copy, scale=rden[:])
                    r0 = b * TC + 64 * ch
                    nc.sync.dma_start(out=out_e[r0 : r0 + 64, :], in_=out_sb[:])
            if PHASE < 9:
                dummy = smallp.tile([64, H], F32, tag="osb")
                nc.vector.tensor_copy(dummy[:], qT_sb[:, 0:H])
                for r in range(0, B * TC, 64):
                    nc.sync.dma_start(out=out_e[r : r + 64, :], in_=dummy[:])
    nc.compile()
    return nc


_CACHE: dict = {}


def _get_nc():
    if "nc" not in _CACHE:
        _CACHE["nc"] = build(NC)
    return _CACHE["nc"]


def _prep_inputs(x, Wq, Wk, Wv, relpos):
    x = np.ascontiguousarray(x, dtype=np.float32)
    relpos = np.ascontiguousarray(relpos, dtype=np.float32)
    wqk = np.ascontiguousarray(
        np.concatenate([Wq, Wk], axis=1), dtype=np.float32
    )
    wv = np.ascontiguousarray(Wv, dtype=np.float32)
    in_maps = []
    for i in range(NC):
        xs = x[:, TC * i : TC * (i + 1), :]            # (B, TC, D)
        xT = np.ascontiguousarray(
            xs.transpose(2, 0, 1).reshape(D, B * TC)
        )
        rp = relpos[TC * i : TC * (i + 1)]             # (TC, T, H)
        relp = np.ascontiguousarray(
            rp.transpose(0, 2, 1).reshape(NPAIR, TC, T)
        ).astype(ml_dtypes.bfloat16)
        tl = np.arange(TC)[:, None]
        vv = np.arange(T)[None, :]
        msk = np.where(vv <= TC * i + tl, 0.0, MASK_VAL).astype(np.float32)
        in_maps.append(
            {"xT": xT, "wqk": wqk, "wv": wv, "relp": relp, "mask": msk}
        )
    return in_maps


def run_sharded(in_maps, trace=False, **kw):
    nc = _get_nc()
    return run_bass_kernel_spmd(
        nc, in_maps, core_ids=list(range(NC)), trace=trace, **kw
    )


def kernel(x, Wq, Wk, Wv, relpos):
    in_maps = _prep_inputs(x, Wq, Wk, Wv, relpos)
    res = run_sharded(in_maps, trace=False)
    out = np.empty((B, T, H), dtype=np.float32)
    for i in range(NC):
        out[:, TC * i : TC * (i + 1), :] = (
            res.results[i]["out"].reshape(B, TC, H)
        )
    return out
